# revision 1
# baseline (speedup 1.0000x reference)
"""Multi-head attention (B=4, S=1024, H=1024, 16 heads) on 8 trn2 cores.

Sharding: 8 shards = (batch b in 0..3) x (head-half hf in 0..1).
Each core computes attention for 8 heads of one batch and a partial
output projection (row-parallel Wo); host sums the two partials per batch.

Per-core pipeline (matmuls in bf16, PSUM fp32, output fp32):
  - QT/KT computed d-major: QT[dq, tok] = Wq_h^T @ x^T  (N=1024 moving)
  - V computed token-major with a ones column appended per head
  - logitsT[k, q] per head via lhsT=KT tile (K=64 contraction),
    exp on ACT with per-partition bias fused (no max-subtraction needed:
    logits are O(+-9) so fp32 exp is exact enough)
  - attn@V: lhsT=V_aug [tok,65] -> psum [65, q]; row 64 = softmax denom
  - normalize via K=1 broadcast matmul of 1/denom + DVE multiply
  - out_partial = attnT^T @ Wo accumulated over d (row-parallel)
"""

import numpy as np
import ml_dtypes

import concourse.bass as bass
import concourse.tile as tile
from concourse import bacc, mybir
from concourse import bass_utils

F32 = mybir.dt.float32
F32R = mybir.dt.float32r
BF16 = mybir.dt.bfloat16
EXP = mybir.ActivationFunctionType.Exp

S = 1024  # sequence length (tokens)
HID = 1024  # model hidden
DQ = 512  # per-core projected dim (8 heads x 64)
NHL = 8  # local heads per core
DH = 64  # head depth
NK = HID // 128  # 8 contraction tiles over hidden
P = 128
N_CORES = 8

MM_DT = BF16  # matmul dtype: BF16 or F32R

_CACHED_NC = None


def _np_in_dt():
    return ml_dtypes.bfloat16 if MM_DT == BF16 else np.float32


def build_program(unroll=1):
    in_dt = MM_DT if MM_DT == BF16 else F32
    nc = bacc.Bacc("TRN2", target_bir_lowering=False, debug=False)
    xt = nc.dram_tensor("xt", [HID, S], in_dt, kind="ExternalInput").ap()
    yt = nc.dram_tensor("yt", [HID, S], in_dt, kind="ExternalInput").ap()
    wq = nc.dram_tensor("wq", [HID, DQ], in_dt, kind="ExternalInput").ap()
    wk = nc.dram_tensor("wk", [HID, DQ], in_dt, kind="ExternalInput").ap()
    wv = nc.dram_tensor("wv", [HID, DQ], in_dt, kind="ExternalInput").ap()
    wo = nc.dram_tensor("wo", [DQ, HID], in_dt, kind="ExternalInput").ap()
    biasd = nc.dram_tensor("biasd", [P, NK], F32, kind="ExternalInput").ap()
    onesd = nc.dram_tensor("onesd", [P, DH], in_dt, kind="ExternalInput").ap()
    out = nc.dram_tensor("out", [S, HID], F32, kind="ExternalOutput").ap()

    with tile.TileContext(nc) as tc:
        for _ in range(unroll):
            emit_kernel(tc, out, xt, yt, wq, wk, wv, wo, biasd, onesd)
    nc.compile()
    return nc


def _ld(ap):
    """DMA source view in the matmul dtype."""
    if MM_DT == F32R:
        return ap.bitcast(F32R)
    return ap


def emit_kernel(tc, out, xt, yt, wq, wk, wv, wo, biasd, onesd):
    nc = tc.nc
    with (
        tc.tile_pool(name="wpool", bufs=1) as wpool,
        tc.tile_pool(name="qkv", bufs=1) as qkvpool,
        tc.tile_pool(name="atp", bufs=1) as atpool,
        tc.tile_pool(name="xypool", bufs=1) as xypool,
    ):
        # ---- input DMA: split across the two HWDGE rings, earliest-needed first
        yt_sb = [None] * NK
        wv_sb = [None] * NK
        xt_sb = [None] * NK
        wq_sb = [None] * NK
        wk_sb = [None] * NK
        for k in range(NK):
            eng = (nc.sync, nc.scalar, nc.gpsimd)[k % 3]
            t = xypool.tile([P, S], MM_DT, tag=f"yt{k}", name=f"yt{k}")
            eng.dma_start(t[:], _ld(yt[k * P : (k + 1) * P, :]))
            yt_sb[k] = t
            t = wpool.tile([P, DQ], MM_DT, tag=f"wv{k}", name=f"wv{k}")
            eng.dma_start(t[:], _ld(wv[k * P : (k + 1) * P, :]))
            wv_sb[k] = t
        bias_sb = wpool.tile([P, NK], F32, tag="bias")
        nc.sync.dma_start(bias_sb[:], biasd[:])
        vones_sb = wpool.tile([P, NHL], MM_DT, tag="vones")
        nc.sync.dma_start(vones_sb[:], _ld(onesd[:, 0:NHL]))
        for k in range(NK):
            eng = (nc.sync, nc.scalar, nc.gpsimd)[k % 3]
            t = xypool.tile([P, S], MM_DT, tag=f"xt{k}", name=f"xt{k}")
            eng.dma_start(t[:], _ld(xt[k * P : (k + 1) * P, :]))
            xt_sb[k] = t
            t = wpool.tile([P, DQ], MM_DT, tag=f"wq{k}", name=f"wq{k}")
            eng.dma_start(t[:], _ld(wq[k * P : (k + 1) * P, :]))
            wq_sb[k] = t
            t = wpool.tile([P, DQ], MM_DT, tag=f"wk{k}", name=f"wk{k}")
            eng.dma_start(t[:], _ld(wk[k * P : (k + 1) * P, :]))
            wk_sb[k] = t
        wo_sb = []
        for k in range(DQ // P):
            t = wpool.tile([P, HID], MM_DT, tag=f"wo{k}", name=f"wo{k}")
            (nc.sync if k % 2 == 0 else nc.scalar).dma_start(
                t[:], _ld(wo[k * P : (k + 1) * P, :])
            )
            wo_sb.append(t)

        # ---- persistent slabs ----
        qt_sb = [qkvpool.tile([P, S], MM_DT, tag=f"qt{m}", name=f"qt{m}") for m in range(4)]
        kt_sb = [qkvpool.tile([P, S], MM_DT, tag=f"kt{m}", name=f"kt{m}") for m in range(4)]
        v_sb = [qkvpool.tile([P, NHL * (DH + 1)], MM_DT, tag=f"v{m}", name=f"v{m}") for m in range(8)]
        at_sb = [atpool.tile([P, S], MM_DT, tag=f"at{m}", name=f"at{m}") for m in range(4)]

        # PSUM pools (8 banks): lg 2x[128,1024]=4, av 1x[65,1024]=2,
        # pjv 2x[128,512]=2 (released before pj 2x[128,512]=2 allocs),
        # wo 3x[128,1024]=6 after everything releases.
        pp_lg = tc.alloc_tile_pool(name="pp_lg", bufs=2, space="PSUM")
        pp_av = tc.alloc_tile_pool(name="pp_av", bufs=1, space="PSUM")
        pp_pjv = tc.alloc_tile_pool(name="pp_pjv", bufs=2, space="PSUM")

        with tc.tile_pool(name="expp", bufs=6) as exppool, tc.tile_pool(
            name="smallp", bufs=4
        ) as smallpool, tc.tile_pool(name="accp", bufs=1) as accpool:
            # ---- V projection (token-major, ones columns appended) ----
            for m in range(8):
                ps = pp_pjv.tile([P, DQ], F32, tag="pjv", name="pjv")
                for k in range(NK):
                    nc.tensor.matmul(
                        ps[:],
                        yt_sb[k][:, m * P : (m + 1) * P],
                        wv_sb[k][:],
                        start=(k == 0),
                        stop=(k == NK - 1),
                    )
                dst3 = v_sb[m][:].rearrange("p (h c) -> p h c", c=DH + 1)
                src3 = ps[:].rearrange("p (h c) -> p h c", c=DH)
                nc.vector.tensor_copy(dst3[:, :, 0:DH], src3[:, :, :])
                nc.vector.tensor_copy(
                    dst3[:, :, DH : DH + 1],
                    vones_sb[:].rearrange("p (a b) -> p a b", b=1),
                )
            pp_pjv.release()
            pp_pj = tc.alloc_tile_pool(name="pp_pj", bufs=1, space="PSUM")
            pp_wo = tc.alloc_tile_pool(name="pp_wo", bufs=1, space="PSUM")

            # ---- software-pipelined: each pair's QT/KT projection is
            # emitted between the previous pair's two heads, so the PE's
            # projection bursts fill time while ACT (the bottleneck)
            # streams exp for the in-flight head.
            def emit_proj_pair(pair):
                for w_sb, src_sb, dst in (
                    (wq_sb, xt_sb, qt_sb),
                    (wk_sb, yt_sb, kt_sb),
                ):
                    for n in range(2):
                        ps = pp_pj.tile([P, 512], F32, tag="pj", name="pj")
                        for k in range(NK):
                            nc.tensor.matmul(
                                ps[:],
                                w_sb[k][:, pair * P : (pair + 1) * P],
                                src_sb[k][:, n * 512 : (n + 1) * 512],
                                start=(k == 0),
                                stop=(k == NK - 1),
                            )
                        nc.vector.tensor_copy(
                            dst[pair][:, n * 512 : (n + 1) * 512], ps[:]
                        )

            def emit_head(h):
                pair, hi = divmod(h, 2)
                base = hi * DH
                av = pp_av.tile([DH + 1, S], F32, tag="av", name="av")
                for sk in range(NK):
                    lg = pp_lg.tile([P, S], F32, tag="lg", name="lg")
                    for n in range(2):
                        nc.tensor.matmul(
                            lg[:, n * 512 : (n + 1) * 512],
                            kt_sb[pair][base : base + DH, sk * P : (sk + 1) * P],
                            qt_sb[pair][base : base + DH, n * 512 : (n + 1) * 512],
                            start=True,
                            stop=True,
                        )
                    e = exppool.tile([P, S], MM_DT, tag="exp", name="exp")
                    nc.scalar.activation(
                        e[:], lg[:], EXP, bias=bias_sb[:, sk : sk + 1]
                    )
                    for n in range(2):
                        nc.tensor.matmul(
                            av[:, n * 512 : (n + 1) * 512],
                            v_sb[sk][:, h * (DH + 1) : (h + 1) * (DH + 1)],
                            e[:, n * 512 : (n + 1) * 512],
                            start=(sk == 0),
                            stop=(sk == NK - 1),
                        )
                # normalize -> attnT slab
                rc = smallpool.tile([1, S], F32, tag="rc", name="rc")
                nc.vector.reciprocal(rc[:], av[DH : DH + 1, :])
                bc_sb = smallpool.tile([DH, S], F32, tag="bcsb", name="bcsb")
                nc.gpsimd.partition_broadcast(bc_sb[:], rc[:])
                nc.vector.tensor_mul(
                    at_sb[pair][base : base + DH, :], av[0:DH, :], bc_sb[:]
                )

            acc_sb = [
                accpool.tile([P, HID], F32, tag=f"acc{m}", name=f"acc{m}")
                for m in range(8)
            ]

            def emit_wo_pair(pair):
                # partial out += at[pair]^T @ wo[pair] into fp32 SBUF accs
                for m in range(8):
                    for n in range(2):
                        ps = pp_wo.tile([P, 512], F32, tag="wops", name="wops")
                        nc.tensor.matmul(
                            ps[:],
                            at_sb[pair][:, m * P : (m + 1) * P],
                            wo_sb[pair][:, n * 512 : (n + 1) * 512],
                            start=True,
                            stop=True,
                        )
                        dst = acc_sb[m][:, n * 512 : (n + 1) * 512]
                        if pair == 0:
                            nc.vector.tensor_copy(dst, ps[:])
                        else:
                            nc.vector.tensor_add(dst, dst, ps[:])

            emit_proj_pair(0)
            for pair in range(4):
                emit_head(2 * pair)
                if pair < 3:
                    emit_proj_pair(pair + 1)
                emit_head(2 * pair + 1)
                emit_wo_pair(pair)
            for m in range(8):
                nc.scalar.dma_start(out[m * P : (m + 1) * P, :], acc_sb[m][:])
            pp_wo.release()
            pp_pj.release()
            pp_av.release()
            pp_lg.release()


def _prep_in_maps(x, y, bias, Wq, Wk, Wv, Wo):
    x = np.asarray(x, dtype=np.float32)
    y = np.asarray(y, dtype=np.float32)
    bias = np.asarray(bias, dtype=np.float32)
    Wq = np.asarray(Wq, dtype=np.float32)
    Wk = np.asarray(Wk, dtype=np.float32)
    Wv = np.asarray(Wv, dtype=np.float32)
    Wo = np.asarray(Wo, dtype=np.float32)
    scale = 1.0 / np.sqrt(DH)
    dt = _np_in_dt()
    in_maps = []
    for c in range(N_CORES):
        b, hf = divmod(c, 2)
        cols = slice(hf * DQ, (hf + 1) * DQ)
        in_maps.append(
            {
                "xt": np.ascontiguousarray(x[b].T).astype(dt),
                "yt": np.ascontiguousarray(y[b].T).astype(dt),
                "wq": np.ascontiguousarray(Wq[:, cols] * scale).astype(dt),
                "wk": np.ascontiguousarray(Wk[:, cols]).astype(dt),
                "wv": np.ascontiguousarray(Wv[:, cols]).astype(dt),
                "wo": np.ascontiguousarray(Wo[cols, :]).astype(dt),
                "biasd": np.ascontiguousarray(bias[b, 0, 0].reshape(NK, P).T),
                "onesd": np.ones((P, DH), dtype=dt),
            }
        )
    return in_maps


def get_program():
    global _CACHED_NC
    if _CACHED_NC is None:
        _CACHED_NC = build_program()
    return _CACHED_NC


def kernel(x, y, bias, Wq, Wk, Wv, Wo):
    nc = get_program()
    in_maps = _prep_in_maps(x, y, bias, Wq, Wk, Wv, Wo)
    res = bass_utils.run_bass_kernel_spmd(nc, in_maps, core_ids=list(range(N_CORES)))
    B = 4
    out = np.empty((B, S, HID), dtype=np.float32)
    for b in range(B):
        out[b] = res.results[2 * b]["out"] + res.results[2 * b + 1]["out"]
    return out

